# revision 32
# baseline (speedup 1.0000x reference)
"""Multi-head self-attention (B=4, T=2048, C=1024, H=16, causal) on 8 TRN2 cores.

Sharding: core = (batch b, head-group hg) with b in 0..3, hg in 0..1.
Each core computes its batch's QKV projection restricted to its 8 heads
(w_qkv column-sharded), full causal attention for those heads, and a
row-sharded partial of the output projection. Host sums the two partials
per batch and adds b_proj.

Dataflow is fully "transposed": activations live as [feature, token] so
every matmul contracts along SBUF partitions and softmax needs no
transposes — attention scores come out as S^T[key, query], softmax
normalization is folded into an extra all-ones column appended to V
(giving sum-of-exp for free), and the normalized O^T[feat, query] feeds
the output projection directly as its moving operand.
"""

import sys
from contextlib import ExitStack

for _p in ("/opt/trn_rl_repo",):
    if _p not in sys.path:
        sys.path.insert(0, _p)

import numpy as np

import concourse.bass as bass
import concourse.bacc as bacc
import concourse.tile as tile
from concourse import mybir
from concourse.bass_utils import run_bass_kernel_spmd

F32 = mybir.dt.float32

B, C, H, HD = 4, 1024, 16, 64
HPC = 8  # heads per core
N_CORES = 8
SCALE = HD ** -0.5
T_FULL = 2048

# Matmul input dtype: float32r streams fp32 operands at 1 cycle/row
# (vs 4 for plain fp32) when the free dim is >= 256.
MM_DT = mybir.dt.float32r


def build_program(T=T_FULL, mm_dt=MM_DT):
    NT = T // 512   # query chunks
    NKB = T // 128  # key blocks
    nc = bacc.Bacc("TRN2", target_bir_lowering=False)
    MD = mm_dt  # dtype for tensors feeding matmuls (float32r rounds at write)

    xT = nc.dram_tensor("xT", [C, T], MD, kind="ExternalInput")
    wqk = nc.dram_tensor("wqk", [C, 1024], MD, kind="ExternalInput")
    wv = nc.dram_tensor("wv", [C, 512], MD, kind="ExternalInput")
    bqk = nc.dram_tensor("bqk", [128, 8], F32, kind="ExternalInput")
    bvb = nc.dram_tensor("bvb", [128, 512], F32, kind="ExternalInput")
    wp = nc.dram_tensor("wp", [512, 1024], MD, kind="ExternalInput")
    masks = nc.dram_tensor("masks", [128, 2048], F32, kind="ExternalInput")
    outT = nc.dram_tensor("outT", [C, T], F32, kind="ExternalOutput")

    with nc.allow_low_precision("float32r matmul pipeline"), \
         tile.TileContext(nc) as tc, ExitStack() as ctx:
        pers = ctx.enter_context(tc.tile_pool(name="pers", bufs=1))
        KT_sb = pers.tile([128, 4 * T], MD, tag="kt", name="KT_sb")
        V_sb = pers.tile([128, NKB * 520], MD, tag="v", name="V_sb")
        Wqk_sb = pers.tile([128, 8 * 1024], MD, tag="wqk", name="Wqk_sb")
        Wv_sb = pers.tile([128, 8 * 512], MD, tag="wv", name="Wv_sb")
        masks_sb = pers.tile([128, 2048], F32, tag="masks", name="masks_sb")
        bqk_sb = pers.tile([128, 8], F32, tag="bqk", name="bqk_sb")
        bvb_sb = pers.tile([128, 512], F32, tag="bvb", name="bvb_sb")
        # all-ones row at partition 64 (engine APs must start 32-aligned;
        # the sum-of-exp lane of attention psum lives at row 64)
        ones_sb = pers.tile([65, 64], F32, tag="ones", name="ones_sb")

        dram = ctx.enter_context(tc.tile_pool(name="dramp", bufs=1, space="DRAM"))
        qtds = [dram.tile([512, 512], MD, tag=f"qtd{t}", name=f"qtd{t}")
                for t in range(NT)]

        mmp = ctx.enter_context(tc.tile_pool(name="mmp", bufs=2, space="PSUM"))
        sp = ctx.enter_context(tc.tile_pool(name="sp", bufs=1, space="PSUM"))
        op = ctx.enter_context(tc.tile_pool(name="op", bufs=1, space="PSUM"))

        xtp = ctx.enter_context(tc.tile_pool(name="xtp", bufs=1))
        qbp = ctx.enter_context(tc.tile_pool(name="qbp", bufs=2))
        qsp = ctx.enter_context(tc.tile_pool(name="qsp", bufs=1))
        otp = ctx.enter_context(tc.tile_pool(name="otp", bufs=2))
        wpp = ctx.enter_context(tc.tile_pool(name="wpp", bufs=3))
        esp = ctx.enter_context(tc.tile_pool(name="esp", bufs=2))
        rsp = ctx.enter_context(tc.tile_pool(name="rsp", bufs=2))
        obp = ctx.enter_context(tc.tile_pool(name="obp", bufs=2))

        nc.vector.memset(ones_sb[:, :], 1.0)
        # col 64 of each 65-wide V head slot must be 1.0 (sum-of-exp lane);
        # fill all of V with 1.0, V evictions overwrite cols 0-63
        nc.vector.memset(V_sb[:, :].bitcast(F32), 1.0)

        nc.sync.dma_start(out=masks_sb[:, :], in_=masks[:, :])
        nc.sync.dma_start(out=bqk_sb[:, :], in_=bqk[:, :])
        nc.sync.dma_start(out=bvb_sb[:, :], in_=bvb[:, :])
        for c in range(8):
            nc.sync.dma_start(
                out=Wqk_sb[:, c * 1024:(c + 1) * 1024],
                in_=wqk[c * 128:(c + 1) * 128, :])
            nc.sync.dma_start(
                out=Wv_sb[:, c * 512:(c + 1) * 512],
                in_=wv[c * 128:(c + 1) * 128, :])

        xT3 = xT.rearrange("(c p) t -> p c t", p=128)
        wp3 = wp.rearrange("(cp p) n -> p cp n", p=128)

        def load_chunk(t):
            xt_all = xtp.tile([128, 8, 512], MD, tag="xt", name="xt_all")
            nc.sync.dma_start(
                out=xt_all[:, :, :], in_=xT3[:, :, t * 512:(t + 1) * 512])
            return xt_all

        def qk_chain(t, m, xt_all):
            ps = mmp.tile([128, 512], F32, tag="mm", name="ps_qk")
            for c in range(8):
                nc.tensor.matmul(
                    out=ps[:, :],
                    lhsT=Wqk_sb[:, c * 1024 + m * 128: c * 1024 + (m + 1) * 128],
                    rhs=xt_all[:, c, :],
                    start=(c == 0), stop=(c == 7))
            if m < 4:
                qb = qbp.tile([128, 512], MD, tag="qb", name="qb")
                nc.vector.tensor_scalar_add(
                    out=qb[:, :], in0=ps[:, :], scalar1=bqk_sb[:, m:m + 1])
                nc.sync.dma_start(
                    out=qtds[t][m * 128:(m + 1) * 128, :], in_=qb[:, :])
            else:
                nc.vector.tensor_scalar_add(
                    out=KT_sb[:, (m - 4) * T + t * 512: (m - 4) * T + (t + 1) * 512],
                    in0=ps[:, :], scalar1=bqk_sb[:, m:m + 1])

        def v_chain(t, tb, xt_all):
            kbg = t * 4 + tb
            ps = mmp.tile([128, 512], F32, tag="mm", name="ps_v")
            for c in range(8):
                nc.tensor.matmul(
                    out=ps[:, :],
                    lhsT=xt_all[:, c, tb * 128:(tb + 1) * 128],
                    rhs=Wv_sb[:, c * 512:(c + 1) * 512],
                    start=(c == 0), stop=(c == 7))
            vdst = V_sb[:, kbg * 520:(kbg + 1) * 520].rearrange(
                "p (h e) -> p h e", e=65)[:, :, 0:64]
            nc.vector.scalar_tensor_tensor(
                out=vdst,
                in0=ps.rearrange("p (h e) -> p h e", e=64),
                scalar=1.0,
                in1=bvb_sb.rearrange("p (h e) -> p h e", e=64),
                op0=mybir.AluOpType.mult,
                op1=mybir.AluOpType.add)

        def chunk_chains(t):
            xt_all = load_chunk(t)
            work = []
            for m in range(8):
                work.append(lambda m=m: qk_chain(t, m, xt_all))
            for tb in range(4):
                work.append(lambda tb=tb: v_chain(t, tb, xt_all))
            return work

        def outproj_chain(j, OT_j, nb):
            wpt = wpp.tile([128, 4, 128], MD, tag="wpt", name="wpt")
            nc.sync.dma_start(
                out=wpt[:, :, :], in_=wp3[:, :, nb * 128:(nb + 1) * 128])
            ps = mmp.tile([128, 512], F32, tag="mm", name="ps_o")
            for cp in range(4):
                nc.tensor.matmul(
                    out=ps[:, :], lhsT=wpt[:, cp, :], rhs=OT_j[:, cp, :],
                    start=(cp == 0), stop=(cp == 3))
            ob = obp.tile([128, 512], F32, tag="ob", name="ob")
            nc.vector.tensor_copy(out=ob[:, :], in_=ps[:, :])
            nc.sync.dma_start(
                out=outT[nb * 128:(nb + 1) * 128, j * 512:(j + 1) * 512],
                in_=ob[:, :])

        def outproj_chains(j, OT_j):
            return [lambda nb=nb: outproj_chain(j, OT_j, nb) for nb in range(8)]

        # ---- prime: projection for query/key chunk 0 ----
        for ch in chunk_chains(0):
            ch()

        # ---- supersteps: attention(j) interleaved with projection(j+1),
        #      output projection(j) as the dense tail of each superstep ----
        prev_ot = None
        for j in range(NT):
            qst = qsp.tile([128, 4, 512], MD, tag="qs", name="qs")
            nc.sync.dma_start(
                out=qst[:, :, :],
                in_=qtds[j].rearrange("(p r) t -> r p t", r=128))
            work = list(chunk_chains(j + 1)) if j + 1 < NT else []
            if prev_ot is not None:
                work += outproj_chains(j - 1, prev_ot)
            wi = 0
            gi = 0
            OT_j = otp.tile([128, 4, 512], MD, tag="ot", name="OT_j")
            for p in range(4):
                rstg = rsp.tile([65, 1024], F32, tag="rstg", name="rstg")
                nkb = 4 * (j + 1)
                poa = op.tile([65, 512], F32, tag="oa", name="poa")
                pob = op.tile([65, 512], F32, tag="ob2", name="pob")
                pos = (poa, pob)
                for G in range(2 * (j + 1)):
                    pssa = sp.tile([128, 1024], F32, tag="sa", name="pssa")
                    pssb = sp.tile([128, 1024], F32, tag="sb", name="pssb")
                    psss = (pssa, pssb)
                    for r in range(2):
                        kb = 2 * G + r
                        # heads of the pair occupy PE row groups 0-63/64-127
                        for parity in range(2):
                            nc.tensor.matmul(
                                out=psss[parity][:, r * 512:(r + 1) * 512],
                                lhsT=KT_sb[64 * parity:64 * (parity + 1),
                                           p * T + kb * 128: p * T + (kb + 1) * 128],
                                rhs=qst[64 * parity:64 * (parity + 1), p, :],
                                start=True, stop=True)
                    ests = []
                    for parity in range(2):
                        est = esp.tile([128, 1024], MD, tag=f"es{parity}",
                                       name=f"est{parity}")
                        nc.scalar.activation(
                            out=est[:, :], in_=psss[parity][:, :],
                            func=mybir.ActivationFunctionType.Exp, scale=SCALE)
                        if G >= 2 * j:
                            nc.gpsimd.tensor_mul(
                                out=est[:, :], in0=est[:, :],
                                in1=masks_sb[:, (G - 2 * j) * 1024:(G - 2 * j + 1) * 1024])
                        ests.append(est)
                    for r in range(2):
                        kb = 2 * G + r
                        for parity in range(2):
                            h = 2 * p + parity
                            nc.tensor.matmul(
                                out=pos[parity][:, :],
                                lhsT=V_sb[:, kb * 520 + h * 65: kb * 520 + (h + 1) * 65],
                                rhs=ests[parity][:, r * 512:(r + 1) * 512],
                                start=(kb == 0), stop=(kb == nkb - 1))
                    # keep the PE fed with dense projection work; paired
                    # chains give ~3.8us uninterrupted bursts, enough to
                    # flip the HAM clock gate back to full rate
                    gi += 1
                    if gi % 4 == 0:
                        for _ in range(4):
                            if wi < len(work):
                                work[wi]()
                                wi += 1
                for parity in range(2):
                    po = pos[parity]
                    # 1/sum_exp for this head (psum row 64 is 32-aligned)
                    nc.vector.reciprocal(
                        out=rstg[64:65, parity * 512:(parity + 1) * 512],
                        in_=po[64:65, :])
                    if parity == 0:
                        nc.vector.tensor_copy(
                            out=OT_j[0:64, p, :], in_=po[0:64, :])
                    else:
                        # odd head rows live at partitions 64-127; engines
                        # cannot partition-shift, bounce via SBUF + DMA
                        stg = obp.tile([64, 512], MD, tag="stg", name="stg")
                        nc.vector.tensor_copy(out=stg[:, :], in_=po[0:64, :])
                        nc.sync.dma_start(
                            out=OT_j[64:128, p, :], in_=stg[:, :])
                # broadcast each head's 1/sum_exp over its 64 O^T rows
                rb = mmp.tile([128, 512], F32, tag="mm", name="rb")
                for parity in range(2):
                    nc.tensor.matmul(
                        out=rb[64 * parity:64 * (parity + 1), :],
                        lhsT=ones_sb[64:65, :],
                        rhs=rstg[64:65, parity * 512:(parity + 1) * 512],
                        start=True, stop=True)
                otn = OT_j[:, p, :]
                nc.vector.tensor_mul(out=otn, in0=otn, in1=rb[:, :])
            while wi < len(work):
                work[wi]()
                wi += 1
            prev_ot = OT_j
        for ch in outproj_chains(NT - 1, prev_ot):
            ch()
    nc.compile()
    return nc


def build_masks():
    # masks[kp, r*512 + qf] = 1.0 iff qf >= kp + 128*r  (r = key-block index
    # within the diagonal group of a 512-wide query chunk)
    kp = np.arange(128)[:, None]
    qf = np.arange(512)[None, :]
    cols = [(qf >= kp + 128 * r).astype(np.float32) for r in range(4)]
    return np.ascontiguousarray(np.concatenate(cols, axis=1))


def make_in_maps(x, w_qkv, b_qkv, w_proj, T=T_FULL):
    x = np.asarray(x, dtype=np.float32)
    w_qkv = np.asarray(w_qkv, dtype=np.float32)
    b_qkv = np.asarray(b_qkv, dtype=np.float32)
    w_proj = np.asarray(w_proj, dtype=np.float32)
    masks = build_masks()
    in_maps = []
    for core in range(N_CORES):
        b, hg = core // 2, core % 2
        qc = slice(hg * 512, hg * 512 + 512)
        kc = slice(1024 + hg * 512, 1024 + hg * 512 + 512)
        vc = slice(2048 + hg * 512, 2048 + hg * 512 + 512)
        bqk_host = np.concatenate([b_qkv[qc], b_qkv[kc]]).reshape(8, 128).T
        in_maps.append({
            "xT": np.ascontiguousarray(x[b, :T, :].T),
            "wqk": np.ascontiguousarray(
                np.concatenate([w_qkv[:, qc], w_qkv[:, kc]], axis=1)),
            "wv": np.ascontiguousarray(w_qkv[:, vc]),
            "bqk": np.ascontiguousarray(bqk_host),
            "bvb": np.ascontiguousarray(
                np.broadcast_to(b_qkv[vc], (128, 512))),
            "wp": np.ascontiguousarray(w_proj[hg * 512:(hg + 1) * 512, :]),
            "masks": masks,
        })
    return in_maps


def assemble_output(results, b_proj, T=T_FULL):
    b_proj = np.asarray(b_proj, dtype=np.float32)
    out = np.empty((B, T, C), dtype=np.float32)
    for b in range(B):
        acc = results[2 * b]["outT"] + results[2 * b + 1]["outT"]
        out[b] = acc.T + b_proj
    return out


_PROG_CACHE = {}


def run(x, w_qkv, b_qkv, w_proj, b_proj, T=T_FULL, mm_dt=MM_DT, trace=False):
    key = (T, str(mm_dt))
    if key not in _PROG_CACHE:
        _PROG_CACHE[key] = build_program(T=T, mm_dt=mm_dt)
    nc = _PROG_CACHE[key]
    in_maps = make_in_maps(x, w_qkv, b_qkv, w_proj, T=T)
    res = run_bass_kernel_spmd(
        nc, in_maps, list(range(N_CORES)), trace=trace,
    )
    out = assemble_output(res.results, b_proj, T=T)
    return out, res


def kernel(x, w_qkv, b_qkv, w_proj, b_proj):
    out, _ = run(x, w_qkv, b_qkv, w_proj, b_proj)
    return out


# revision 33
# speedup vs baseline: 1.0180x; 1.0180x over previous
"""Multi-head self-attention (B=4, T=2048, C=1024, H=16, causal) on 8 TRN2 cores.

Sharding: core = (batch b, head-group hg) with b in 0..3, hg in 0..1.
Each core computes its batch's QKV projection restricted to its 8 heads
(w_qkv column-sharded), full causal attention for those heads, and a
row-sharded partial of the output projection. Host sums the two partials
per batch and adds b_proj.

Dataflow is fully "transposed": activations live as [feature, token] so
every matmul contracts along SBUF partitions and softmax needs no
transposes — attention scores come out as S^T[key, query], softmax
normalization is folded into an extra all-ones column appended to V
(giving sum-of-exp for free), and the normalized O^T[feat, query] feeds
the output projection directly as its moving operand.
"""

import sys
from contextlib import ExitStack

for _p in ("/opt/trn_rl_repo",):
    if _p not in sys.path:
        sys.path.insert(0, _p)

import numpy as np

import concourse.bass as bass
import concourse.bacc as bacc
import concourse.tile as tile
from concourse import mybir
from concourse.bass_utils import run_bass_kernel_spmd

F32 = mybir.dt.float32

B, C, H, HD = 4, 1024, 16, 64
HPC = 8  # heads per core
N_CORES = 8
SCALE = HD ** -0.5
T_FULL = 2048

# Matmul input dtype: float32r streams fp32 operands at 1 cycle/row
# (vs 4 for plain fp32) when the free dim is >= 256.
MM_DT = mybir.dt.float32r


def build_program(T=T_FULL, mm_dt=MM_DT):
    NT = T // 512   # query chunks
    NKB = T // 128  # key blocks
    nc = bacc.Bacc("TRN2", target_bir_lowering=False)
    MD = mm_dt  # dtype for tensors feeding matmuls (float32r rounds at write)

    xT = nc.dram_tensor("xT", [C, T], MD, kind="ExternalInput")
    wqk = nc.dram_tensor("wqk", [C, 1024], MD, kind="ExternalInput")
    wv = nc.dram_tensor("wv", [C, 512], MD, kind="ExternalInput")
    bqk = nc.dram_tensor("bqk", [128, 8], F32, kind="ExternalInput")
    bvb = nc.dram_tensor("bvb", [128, 512], F32, kind="ExternalInput")
    wp = nc.dram_tensor("wp", [512, 1024], MD, kind="ExternalInput")
    masks = nc.dram_tensor("masks", [128, 2048], F32, kind="ExternalInput")
    outT = nc.dram_tensor("outT", [C, T], F32, kind="ExternalOutput")

    with nc.allow_low_precision("float32r matmul pipeline"), \
         tile.TileContext(nc) as tc, ExitStack() as ctx:
        pers = ctx.enter_context(tc.tile_pool(name="pers", bufs=1))
        KT_sb = pers.tile([128, 4 * T], MD, tag="kt", name="KT_sb")
        V_sb = pers.tile([128, NKB * 520], MD, tag="v", name="V_sb")
        Wqk_sb = pers.tile([128, 8 * 1024], MD, tag="wqk", name="Wqk_sb")
        Wv_sb = pers.tile([128, 8 * 512], MD, tag="wv", name="Wv_sb")
        masks_sb = pers.tile([128, 2048], F32, tag="masks", name="masks_sb")
        bqk_sb = pers.tile([128, 8], F32, tag="bqk", name="bqk_sb")
        bvb_sb = pers.tile([128, 512], F32, tag="bvb", name="bvb_sb")
        # all-ones row at partition 64 (engine APs must start 32-aligned;
        # the sum-of-exp lane of attention psum lives at row 64)
        ones_sb = pers.tile([65, 64], F32, tag="ones", name="ones_sb")

        dram = ctx.enter_context(tc.tile_pool(name="dramp", bufs=1, space="DRAM"))
        qtds = [dram.tile([512, 512], MD, tag=f"qtd{t}", name=f"qtd{t}")
                for t in range(NT)]

        mmp = ctx.enter_context(tc.tile_pool(name="mmp", bufs=2, space="PSUM"))
        sp = ctx.enter_context(tc.tile_pool(name="sp", bufs=1, space="PSUM"))
        op = ctx.enter_context(tc.tile_pool(name="op", bufs=1, space="PSUM"))

        xtp = ctx.enter_context(tc.tile_pool(name="xtp", bufs=1))
        qbp = ctx.enter_context(tc.tile_pool(name="qbp", bufs=2))
        qsp = ctx.enter_context(tc.tile_pool(name="qsp", bufs=1))
        otp = ctx.enter_context(tc.tile_pool(name="otp", bufs=2))
        wpp = ctx.enter_context(tc.tile_pool(name="wpp", bufs=3))
        esp = ctx.enter_context(tc.tile_pool(name="esp", bufs=2))
        rsp = ctx.enter_context(tc.tile_pool(name="rsp", bufs=2))
        obp = ctx.enter_context(tc.tile_pool(name="obp", bufs=2))

        nc.vector.memset(ones_sb[:, :], 1.0)
        # col 64 of each 65-wide V head slot must be 1.0 (sum-of-exp lane);
        # fill all of V with 1.0, V evictions overwrite cols 0-63
        nc.vector.memset(V_sb[:, :].bitcast(F32), 1.0)

        nc.sync.dma_start(out=masks_sb[:, :], in_=masks[:, :])
        nc.sync.dma_start(out=bqk_sb[:, :], in_=bqk[:, :])
        nc.sync.dma_start(out=bvb_sb[:, :], in_=bvb[:, :])
        for c in range(8):
            nc.sync.dma_start(
                out=Wqk_sb[:, c * 1024:(c + 1) * 1024],
                in_=wqk[c * 128:(c + 1) * 128, :])
            nc.sync.dma_start(
                out=Wv_sb[:, c * 512:(c + 1) * 512],
                in_=wv[c * 128:(c + 1) * 128, :])

        xT3 = xT.rearrange("(c p) t -> p c t", p=128)
        wp3 = wp.rearrange("(cp p) n -> p cp n", p=128)

        def load_chunk(t):
            xt_all = xtp.tile([128, 8, 512], MD, tag="xt", name="xt_all")
            nc.sync.dma_start(
                out=xt_all[:, :, :], in_=xT3[:, :, t * 512:(t + 1) * 512])
            return xt_all

        def qk_chain(t, m, xt_all):
            ps = mmp.tile([128, 512], F32, tag="mm", name="ps_qk")
            for c in range(8):
                nc.tensor.matmul(
                    out=ps[:, :],
                    lhsT=Wqk_sb[:, c * 1024 + m * 128: c * 1024 + (m + 1) * 128],
                    rhs=xt_all[:, c, :],
                    start=(c == 0), stop=(c == 7))
            if m < 4:
                qb = qbp.tile([128, 512], MD, tag="qb", name="qb")
                nc.vector.tensor_scalar_add(
                    out=qb[:, :], in0=ps[:, :], scalar1=bqk_sb[:, m:m + 1])
                nc.sync.dma_start(
                    out=qtds[t][m * 128:(m + 1) * 128, :], in_=qb[:, :])
            else:
                nc.vector.tensor_scalar_add(
                    out=KT_sb[:, (m - 4) * T + t * 512: (m - 4) * T + (t + 1) * 512],
                    in0=ps[:, :], scalar1=bqk_sb[:, m:m + 1])

        def v_chain(t, tb, xt_all):
            kbg = t * 4 + tb
            ps = mmp.tile([128, 512], F32, tag="mm", name="ps_v")
            for c in range(8):
                nc.tensor.matmul(
                    out=ps[:, :],
                    lhsT=xt_all[:, c, tb * 128:(tb + 1) * 128],
                    rhs=Wv_sb[:, c * 512:(c + 1) * 512],
                    start=(c == 0), stop=(c == 7))
            vdst = V_sb[:, kbg * 520:(kbg + 1) * 520].rearrange(
                "p (h e) -> p h e", e=65)[:, :, 0:64]
            nc.vector.scalar_tensor_tensor(
                out=vdst,
                in0=ps.rearrange("p (h e) -> p h e", e=64),
                scalar=1.0,
                in1=bvb_sb.rearrange("p (h e) -> p h e", e=64),
                op0=mybir.AluOpType.mult,
                op1=mybir.AluOpType.add)

        def chunk_chains(t):
            xt_all = load_chunk(t)
            work = []
            for m in range(8):
                work.append(lambda m=m: qk_chain(t, m, xt_all))
            for tb in range(4):
                work.append(lambda tb=tb: v_chain(t, tb, xt_all))
            return work

        def outproj_chain(j, OT_j, nb):
            wpt = wpp.tile([128, 4, 128], MD, tag="wpt", name="wpt")
            nc.sync.dma_start(
                out=wpt[:, :, :], in_=wp3[:, :, nb * 128:(nb + 1) * 128])
            ps = mmp.tile([128, 512], F32, tag="mm", name="ps_o")
            for cp in range(4):
                nc.tensor.matmul(
                    out=ps[:, :], lhsT=wpt[:, cp, :], rhs=OT_j[:, cp, :],
                    start=(cp == 0), stop=(cp == 3))
            ob = obp.tile([128, 512], F32, tag="ob", name="ob")
            nc.vector.tensor_copy(out=ob[:, :], in_=ps[:, :])
            nc.sync.dma_start(
                out=outT[nb * 128:(nb + 1) * 128, j * 512:(j + 1) * 512],
                in_=ob[:, :])

        def outproj_chains(j, OT_j):
            return [lambda nb=nb: outproj_chain(j, OT_j, nb) for nb in range(8)]

        # ---- prime: projection for query/key chunk 0 ----
        for ch in chunk_chains(0):
            ch()

        # ---- supersteps: attention(j) interleaved with projection(j+1),
        #      output projection(j) as the dense tail of each superstep ----
        prev_ot = None
        for j in range(NT):
            qst = qsp.tile([128, 4, 512], MD, tag="qs", name="qs")
            nc.sync.dma_start(
                out=qst[:, :, :],
                in_=qtds[j].rearrange("(p r) t -> r p t", r=128))
            work = list(chunk_chains(j + 1)) if j + 1 < NT else []
            if prev_ot is not None:
                work += outproj_chains(j - 1, prev_ot)
            wi = 0
            gi = 0
            OT_j = otp.tile([128, 4, 512], MD, tag="ot", name="OT_j")
            for p in range(4):
                rstg = rsp.tile([65, 1024], F32, tag="rstg", name="rstg")
                nkb = 4 * (j + 1)
                poa = op.tile([65, 512], F32, tag="oa", name="poa")
                pob = op.tile([65, 512], F32, tag="ob2", name="pob")
                pos = (poa, pob)
                for G in range(2 * (j + 1)):
                    pssa = sp.tile([128, 1024], F32, tag="sa", name="pssa")
                    pssb = sp.tile([128, 1024], F32, tag="sb", name="pssb")
                    psss = (pssa, pssb)
                    for r in range(2):
                        kb = 2 * G + r
                        # heads of the pair occupy PE row groups 0-63/64-127
                        for parity in range(2):
                            nc.tensor.matmul(
                                out=psss[parity][:, r * 512:(r + 1) * 512],
                                lhsT=KT_sb[64 * parity:64 * (parity + 1),
                                           p * T + kb * 128: p * T + (kb + 1) * 128],
                                rhs=qst[64 * parity:64 * (parity + 1), p, :],
                                start=True, stop=True)
                    ests = []
                    for parity in range(2):
                        est = esp.tile([128, 1024], MD, tag=f"es{parity}",
                                       name=f"est{parity}")
                        nc.scalar.activation(
                            out=est[:, :], in_=psss[parity][:, :],
                            func=mybir.ActivationFunctionType.Exp, scale=SCALE)
                        if G >= 2 * j:
                            nc.gpsimd.tensor_mul(
                                out=est[:, :], in0=est[:, :],
                                in1=masks_sb[:, (G - 2 * j) * 1024:(G - 2 * j + 1) * 1024])
                        ests.append(est)
                    for r in range(2):
                        kb = 2 * G + r
                        for parity in range(2):
                            h = 2 * p + parity
                            nc.tensor.matmul(
                                out=pos[parity][:, :],
                                lhsT=V_sb[:, kb * 520 + h * 65: kb * 520 + (h + 1) * 65],
                                rhs=ests[parity][:, r * 512:(r + 1) * 512],
                                start=(kb == 0), stop=(kb == nkb - 1))
                    # keep the PE fed with dense projection work; paired
                    # chains give ~3.8us uninterrupted bursts, enough to
                    # flip the HAM clock gate back to full rate
                    gi += 1
                    if gi % 2 == 0:
                        for _ in range(2):
                            if wi < len(work):
                                work[wi]()
                                wi += 1
                for parity in range(2):
                    po = pos[parity]
                    # 1/sum_exp for this head (psum row 64 is 32-aligned)
                    nc.vector.reciprocal(
                        out=rstg[64:65, parity * 512:(parity + 1) * 512],
                        in_=po[64:65, :])
                    if parity == 0:
                        nc.vector.tensor_copy(
                            out=OT_j[0:64, p, :], in_=po[0:64, :])
                    else:
                        # odd head rows live at partitions 64-127; engines
                        # cannot partition-shift, bounce via SBUF + DMA
                        stg = obp.tile([64, 512], MD, tag="stg", name="stg")
                        nc.vector.tensor_copy(out=stg[:, :], in_=po[0:64, :])
                        nc.sync.dma_start(
                            out=OT_j[64:128, p, :], in_=stg[:, :])
                # broadcast each head's 1/sum_exp over its 64 O^T rows
                rb = mmp.tile([128, 512], F32, tag="mm", name="rb")
                for parity in range(2):
                    nc.tensor.matmul(
                        out=rb[64 * parity:64 * (parity + 1), :],
                        lhsT=ones_sb[64:65, :],
                        rhs=rstg[64:65, parity * 512:(parity + 1) * 512],
                        start=True, stop=True)
                otn = OT_j[:, p, :]
                nc.vector.tensor_mul(out=otn, in0=otn, in1=rb[:, :])
            while wi < len(work):
                work[wi]()
                wi += 1
            prev_ot = OT_j
        for ch in outproj_chains(NT - 1, prev_ot):
            ch()
    nc.compile()
    return nc


def build_masks():
    # masks[kp, r*512 + qf] = 1.0 iff qf >= kp + 128*r  (r = key-block index
    # within the diagonal group of a 512-wide query chunk)
    kp = np.arange(128)[:, None]
    qf = np.arange(512)[None, :]
    cols = [(qf >= kp + 128 * r).astype(np.float32) for r in range(4)]
    return np.ascontiguousarray(np.concatenate(cols, axis=1))


def make_in_maps(x, w_qkv, b_qkv, w_proj, T=T_FULL):
    x = np.asarray(x, dtype=np.float32)
    w_qkv = np.asarray(w_qkv, dtype=np.float32)
    b_qkv = np.asarray(b_qkv, dtype=np.float32)
    w_proj = np.asarray(w_proj, dtype=np.float32)
    masks = build_masks()
    in_maps = []
    for core in range(N_CORES):
        b, hg = core // 2, core % 2
        qc = slice(hg * 512, hg * 512 + 512)
        kc = slice(1024 + hg * 512, 1024 + hg * 512 + 512)
        vc = slice(2048 + hg * 512, 2048 + hg * 512 + 512)
        bqk_host = np.concatenate([b_qkv[qc], b_qkv[kc]]).reshape(8, 128).T
        in_maps.append({
            "xT": np.ascontiguousarray(x[b, :T, :].T),
            "wqk": np.ascontiguousarray(
                np.concatenate([w_qkv[:, qc], w_qkv[:, kc]], axis=1)),
            "wv": np.ascontiguousarray(w_qkv[:, vc]),
            "bqk": np.ascontiguousarray(bqk_host),
            "bvb": np.ascontiguousarray(
                np.broadcast_to(b_qkv[vc], (128, 512))),
            "wp": np.ascontiguousarray(w_proj[hg * 512:(hg + 1) * 512, :]),
            "masks": masks,
        })
    return in_maps


def assemble_output(results, b_proj, T=T_FULL):
    b_proj = np.asarray(b_proj, dtype=np.float32)
    out = np.empty((B, T, C), dtype=np.float32)
    for b in range(B):
        acc = results[2 * b]["outT"] + results[2 * b + 1]["outT"]
        out[b] = acc.T + b_proj
    return out


_PROG_CACHE = {}


def run(x, w_qkv, b_qkv, w_proj, b_proj, T=T_FULL, mm_dt=MM_DT, trace=False):
    key = (T, str(mm_dt))
    if key not in _PROG_CACHE:
        _PROG_CACHE[key] = build_program(T=T, mm_dt=mm_dt)
    nc = _PROG_CACHE[key]
    in_maps = make_in_maps(x, w_qkv, b_qkv, w_proj, T=T)
    res = run_bass_kernel_spmd(
        nc, in_maps, list(range(N_CORES)), trace=trace,
    )
    out = assemble_output(res.results, b_proj, T=T)
    return out, res


def kernel(x, w_qkv, b_qkv, w_proj, b_proj):
    out, _ = run(x, w_qkv, b_qkv, w_proj, b_proj)
    return out


# revision 34
# speedup vs baseline: 1.1124x; 1.0927x over previous
"""Multi-head self-attention (B=4, T=2048, C=1024, H=16, causal) on 8 TRN2 cores.

Sharding: core = (batch b, head-group hg) with b in 0..3, hg in 0..1.
Each core computes its batch's QKV projection restricted to its 8 heads
(w_qkv column-sharded), full causal attention for those heads, and a
row-sharded partial of the output projection. Host sums the two partials
per batch and adds b_proj.

Dataflow is fully "transposed": activations live as [feature, token] so
every matmul contracts along SBUF partitions and softmax needs no
transposes — attention scores come out as S^T[key, query], softmax
normalization is folded into an extra all-ones column appended to V
(giving sum-of-exp for free), and the normalized O^T[feat, query] feeds
the output projection directly as its moving operand.
"""

import sys
from contextlib import ExitStack

for _p in ("/opt/trn_rl_repo",):
    if _p not in sys.path:
        sys.path.insert(0, _p)

import numpy as np

import concourse.bass as bass
import concourse.bacc as bacc
import concourse.tile as tile
from concourse import mybir
from concourse.bass_utils import run_bass_kernel_spmd

F32 = mybir.dt.float32

B, C, H, HD = 4, 1024, 16, 64
HPC = 8  # heads per core
N_CORES = 8
SCALE = HD ** -0.5
T_FULL = 2048

# Matmul input dtype: float32r streams fp32 operands at 1 cycle/row
# (vs 4 for plain fp32) when the free dim is >= 256.
MM_DT = mybir.dt.float32r


def build_program(T=T_FULL, mm_dt=MM_DT):
    NT = T // 512   # query chunks
    NKB = T // 128  # key blocks
    nc = bacc.Bacc("TRN2", target_bir_lowering=False)
    MD = mm_dt  # dtype for tensors feeding matmuls (float32r rounds at write)

    xT = nc.dram_tensor("xT", [C, T], MD, kind="ExternalInput")
    wqk = nc.dram_tensor("wqk", [C, 1024], MD, kind="ExternalInput")
    wv = nc.dram_tensor("wv", [C, 512], MD, kind="ExternalInput")
    bqk = nc.dram_tensor("bqk", [128, 8], F32, kind="ExternalInput")
    bvb = nc.dram_tensor("bvb", [128, 512], F32, kind="ExternalInput")
    wp = nc.dram_tensor("wp", [512, 1024], MD, kind="ExternalInput")
    masks = nc.dram_tensor("masks", [128, 2048], F32, kind="ExternalInput")
    outT = nc.dram_tensor("outT", [C, T], F32, kind="ExternalOutput")

    with nc.allow_low_precision("float32r matmul pipeline"), \
         tile.TileContext(nc) as tc, ExitStack() as ctx:
        pers = ctx.enter_context(tc.tile_pool(name="pers", bufs=1))
        KT_sb = pers.tile([128, 4 * T], MD, tag="kt", name="KT_sb")
        V_sb = pers.tile([128, NKB * 520], MD, tag="v", name="V_sb")
        Wqk_sb = pers.tile([128, 8 * 1024], MD, tag="wqk", name="Wqk_sb")
        Wv_sb = pers.tile([128, 8 * 512], MD, tag="wv", name="Wv_sb")
        masks_sb = pers.tile([128, 2048], F32, tag="masks", name="masks_sb")
        bqk_sb = pers.tile([128, 8], F32, tag="bqk", name="bqk_sb")
        bvb_sb = pers.tile([128, 512], F32, tag="bvb", name="bvb_sb")
        # all-ones row at partition 64 (engine APs must start 32-aligned;
        # the sum-of-exp lane of attention psum lives at row 64)
        ones_sb = pers.tile([65, 64], F32, tag="ones", name="ones_sb")

        dram = ctx.enter_context(tc.tile_pool(name="dramp", bufs=1, space="DRAM"))
        qtds = [dram.tile([512, 512], MD, tag=f"qtd{t}", name=f"qtd{t}")
                for t in range(NT)]

        mmp = ctx.enter_context(tc.tile_pool(name="mmp", bufs=2, space="PSUM"))
        sp = ctx.enter_context(tc.tile_pool(name="sp", bufs=1, space="PSUM"))
        op = ctx.enter_context(tc.tile_pool(name="op", bufs=1, space="PSUM"))

        xtp = ctx.enter_context(tc.tile_pool(name="xtp", bufs=1))
        qbp = ctx.enter_context(tc.tile_pool(name="qbp", bufs=2))
        qsp = ctx.enter_context(tc.tile_pool(name="qsp", bufs=1))
        otp = ctx.enter_context(tc.tile_pool(name="otp", bufs=2))
        wpp = ctx.enter_context(tc.tile_pool(name="wpp", bufs=3))
        esp = ctx.enter_context(tc.tile_pool(name="esp", bufs=2))
        rsp = ctx.enter_context(tc.tile_pool(name="rsp", bufs=2))
        obp = ctx.enter_context(tc.tile_pool(name="obp", bufs=2))

        nc.vector.memset(ones_sb[:, :], 1.0)
        # col 64 of each 65-wide V head slot must be 1.0 (sum-of-exp lane);
        # fill all of V with 1.0, V evictions overwrite cols 0-63
        nc.vector.memset(V_sb[:, :].bitcast(F32), 1.0)

        nc.sync.dma_start(out=masks_sb[:, :], in_=masks[:, :])
        nc.sync.dma_start(out=bqk_sb[:, :], in_=bqk[:, :])
        nc.sync.dma_start(out=bvb_sb[:, :], in_=bvb[:, :])
        for c in range(8):
            nc.sync.dma_start(
                out=Wqk_sb[:, c * 1024:(c + 1) * 1024],
                in_=wqk[c * 128:(c + 1) * 128, :])
            nc.sync.dma_start(
                out=Wv_sb[:, c * 512:(c + 1) * 512],
                in_=wv[c * 128:(c + 1) * 128, :])

        xT3 = xT.rearrange("(c p) t -> p c t", p=128)
        wp3 = wp.rearrange("(cp p) n -> p cp n", p=128)

        def load_chunk(t):
            xt_all = xtp.tile([128, 8, 512], MD, tag="xt", name="xt_all")
            nc.sync.dma_start(
                out=xt_all[:, :, :], in_=xT3[:, :, t * 512:(t + 1) * 512])
            return xt_all

        def qk_chain(t, m, xt_all):
            ps = mmp.tile([128, 512], F32, tag="mm", name="ps_qk")
            for c in range(8):
                nc.tensor.matmul(
                    out=ps[:, :],
                    lhsT=Wqk_sb[:, c * 1024 + m * 128: c * 1024 + (m + 1) * 128],
                    rhs=xt_all[:, c, :],
                    start=(c == 0), stop=(c == 7))
            if m < 4:
                qb = qbp.tile([128, 512], MD, tag="qb", name="qb")
                nc.vector.tensor_scalar_add(
                    out=qb[:, :], in0=ps[:, :], scalar1=bqk_sb[:, m:m + 1])
                nc.sync.dma_start(
                    out=qtds[t][m * 128:(m + 1) * 128, :], in_=qb[:, :])
            else:
                nc.vector.tensor_scalar_add(
                    out=KT_sb[:, (m - 4) * T + t * 512: (m - 4) * T + (t + 1) * 512],
                    in0=ps[:, :], scalar1=bqk_sb[:, m:m + 1])

        def v_chain(t, tb, xt_all):
            kbg = t * 4 + tb
            ps = mmp.tile([128, 512], F32, tag="mm", name="ps_v")
            for c in range(8):
                nc.tensor.matmul(
                    out=ps[:, :],
                    lhsT=xt_all[:, c, tb * 128:(tb + 1) * 128],
                    rhs=Wv_sb[:, c * 512:(c + 1) * 512],
                    start=(c == 0), stop=(c == 7))
            vdst = V_sb[:, kbg * 520:(kbg + 1) * 520].rearrange(
                "p (h e) -> p h e", e=65)[:, :, 0:64]
            nc.vector.scalar_tensor_tensor(
                out=vdst,
                in0=ps.rearrange("p (h e) -> p h e", e=64),
                scalar=1.0,
                in1=bvb_sb.rearrange("p (h e) -> p h e", e=64),
                op0=mybir.AluOpType.mult,
                op1=mybir.AluOpType.add)

        def chunk_chains(t):
            xt_all = load_chunk(t)
            work = []
            for m in range(8):
                work.append(lambda m=m: qk_chain(t, m, xt_all))
            for tb in range(4):
                work.append(lambda tb=tb: v_chain(t, tb, xt_all))
            return work

        def outproj_chain(j, OT_j, nb):
            wpt = wpp.tile([128, 4, 128], MD, tag="wpt", name="wpt")
            nc.sync.dma_start(
                out=wpt[:, :, :], in_=wp3[:, :, nb * 128:(nb + 1) * 128])
            ps = mmp.tile([128, 512], F32, tag="mm", name="ps_o")
            for cp in range(4):
                nc.tensor.matmul(
                    out=ps[:, :], lhsT=wpt[:, cp, :], rhs=OT_j[:, cp, :],
                    start=(cp == 0), stop=(cp == 3))
            ob = obp.tile([128, 512], F32, tag="ob", name="ob")
            nc.vector.tensor_copy(out=ob[:, :], in_=ps[:, :])
            nc.sync.dma_start(
                out=outT[nb * 128:(nb + 1) * 128, j * 512:(j + 1) * 512],
                in_=ob[:, :])

        def outproj_chains(j, OT_j):
            return [lambda nb=nb: outproj_chain(j, OT_j, nb) for nb in range(8)]

        # ---- prime: projection for query/key chunk 0 ----
        for ch in chunk_chains(0):
            ch()

        # ---- supersteps: attention(j) interleaved with projection(j+1),
        #      output projection(j) as the dense tail of each superstep ----
        prev_ot = None
        for j in range(NT):
            qst = qsp.tile([128, 4, 512], MD, tag="qs", name="qs")
            nc.sync.dma_start(
                out=qst[:, :, :],
                in_=qtds[j].rearrange("(p r) t -> r p t", r=128))
            work = list(chunk_chains(j + 1)) if j + 1 < NT else []
            if prev_ot is not None:
                work += outproj_chains(j - 1, prev_ot)
            wi = 0
            gi = 0
            OT_j = otp.tile([128, 4, 512], MD, tag="ot", name="OT_j")
            for p in range(4):
                rstg = rsp.tile([65, 1024], F32, tag="rstg", name="rstg")
                nkb = 4 * (j + 1)
                poa = op.tile([65, 512], F32, tag="oa", name="poa")
                pob = op.tile([65, 512], F32, tag="ob2", name="pob")
                pos = (poa, pob)
                for G in range(2 * (j + 1)):
                    pssa = sp.tile([128, 1024], F32, tag="sa", name="pssa")
                    pssb = sp.tile([128, 1024], F32, tag="sb", name="pssb")
                    psss = (pssa, pssb)
                    for r in range(2):
                        kb = 2 * G + r
                        # heads of the pair occupy PE row groups 0-63/64-127
                        for parity in range(2):
                            nc.tensor.matmul(
                                out=psss[parity][:, r * 512:(r + 1) * 512],
                                lhsT=KT_sb[64 * parity:64 * (parity + 1),
                                           p * T + kb * 128: p * T + (kb + 1) * 128],
                                rhs=qst[64 * parity:64 * (parity + 1), p, :],
                                start=True, stop=True)
                    ests = []
                    for parity in range(2):
                        est = esp.tile([128, 1024], MD, tag=f"es{parity}",
                                       name=f"est{parity}")
                        nc.scalar.activation(
                            out=est[:, :], in_=psss[parity][:, :],
                            func=mybir.ActivationFunctionType.Exp, scale=SCALE)
                        if G >= 2 * j:
                            nc.vector.tensor_mul(
                                out=est[:, :], in0=est[:, :],
                                in1=masks_sb[:, (G - 2 * j) * 1024:(G - 2 * j + 1) * 1024])
                        ests.append(est)
                    for r in range(2):
                        kb = 2 * G + r
                        for parity in range(2):
                            h = 2 * p + parity
                            nc.tensor.matmul(
                                out=pos[parity][:, :],
                                lhsT=V_sb[:, kb * 520 + h * 65: kb * 520 + (h + 1) * 65],
                                rhs=ests[parity][:, r * 512:(r + 1) * 512],
                                start=(kb == 0), stop=(kb == nkb - 1))
                    # keep the PE fed with dense projection work; paired
                    # chains give ~3.8us uninterrupted bursts, enough to
                    # flip the HAM clock gate back to full rate
                    gi += 1
                    if gi % 2 == 0:
                        for _ in range(2):
                            if wi < len(work):
                                work[wi]()
                                wi += 1
                for parity in range(2):
                    po = pos[parity]
                    # 1/sum_exp for this head (psum row 64 is 32-aligned)
                    nc.vector.reciprocal(
                        out=rstg[64:65, parity * 512:(parity + 1) * 512],
                        in_=po[64:65, :])
                    if parity == 0:
                        nc.vector.tensor_copy(
                            out=OT_j[0:64, p, :], in_=po[0:64, :])
                    else:
                        # odd head rows live at partitions 64-127; engines
                        # cannot partition-shift, bounce via SBUF + DMA
                        stg = obp.tile([64, 512], MD, tag="stg", name="stg")
                        nc.vector.tensor_copy(out=stg[:, :], in_=po[0:64, :])
                        nc.sync.dma_start(
                            out=OT_j[64:128, p, :], in_=stg[:, :])
                # broadcast each head's 1/sum_exp over its 64 O^T rows
                rb = mmp.tile([128, 512], F32, tag="mm", name="rb")
                for parity in range(2):
                    nc.tensor.matmul(
                        out=rb[64 * parity:64 * (parity + 1), :],
                        lhsT=ones_sb[64:65, :],
                        rhs=rstg[64:65, parity * 512:(parity + 1) * 512],
                        start=True, stop=True)
                otn = OT_j[:, p, :]
                nc.vector.tensor_mul(out=otn, in0=otn, in1=rb[:, :])
            while wi < len(work):
                work[wi]()
                wi += 1
            prev_ot = OT_j
        for ch in outproj_chains(NT - 1, prev_ot):
            ch()
    nc.compile()
    return nc


def build_masks():
    # masks[kp, r*512 + qf] = 1.0 iff qf >= kp + 128*r  (r = key-block index
    # within the diagonal group of a 512-wide query chunk)
    kp = np.arange(128)[:, None]
    qf = np.arange(512)[None, :]
    cols = [(qf >= kp + 128 * r).astype(np.float32) for r in range(4)]
    return np.ascontiguousarray(np.concatenate(cols, axis=1))


def make_in_maps(x, w_qkv, b_qkv, w_proj, T=T_FULL):
    x = np.asarray(x, dtype=np.float32)
    w_qkv = np.asarray(w_qkv, dtype=np.float32)
    b_qkv = np.asarray(b_qkv, dtype=np.float32)
    w_proj = np.asarray(w_proj, dtype=np.float32)
    masks = build_masks()
    in_maps = []
    for core in range(N_CORES):
        b, hg = core // 2, core % 2
        qc = slice(hg * 512, hg * 512 + 512)
        kc = slice(1024 + hg * 512, 1024 + hg * 512 + 512)
        vc = slice(2048 + hg * 512, 2048 + hg * 512 + 512)
        bqk_host = np.concatenate([b_qkv[qc], b_qkv[kc]]).reshape(8, 128).T
        in_maps.append({
            "xT": np.ascontiguousarray(x[b, :T, :].T),
            "wqk": np.ascontiguousarray(
                np.concatenate([w_qkv[:, qc], w_qkv[:, kc]], axis=1)),
            "wv": np.ascontiguousarray(w_qkv[:, vc]),
            "bqk": np.ascontiguousarray(bqk_host),
            "bvb": np.ascontiguousarray(
                np.broadcast_to(b_qkv[vc], (128, 512))),
            "wp": np.ascontiguousarray(w_proj[hg * 512:(hg + 1) * 512, :]),
            "masks": masks,
        })
    return in_maps


def assemble_output(results, b_proj, T=T_FULL):
    b_proj = np.asarray(b_proj, dtype=np.float32)
    out = np.empty((B, T, C), dtype=np.float32)
    for b in range(B):
        acc = results[2 * b]["outT"] + results[2 * b + 1]["outT"]
        out[b] = acc.T + b_proj
    return out


_PROG_CACHE = {}


def run(x, w_qkv, b_qkv, w_proj, b_proj, T=T_FULL, mm_dt=MM_DT, trace=False):
    key = (T, str(mm_dt))
    if key not in _PROG_CACHE:
        _PROG_CACHE[key] = build_program(T=T, mm_dt=mm_dt)
    nc = _PROG_CACHE[key]
    in_maps = make_in_maps(x, w_qkv, b_qkv, w_proj, T=T)
    res = run_bass_kernel_spmd(
        nc, in_maps, list(range(N_CORES)), trace=trace,
    )
    out = assemble_output(res.results, b_proj, T=T)
    return out, res


def kernel(x, w_qkv, b_qkv, w_proj, b_proj):
    out, _ = run(x, w_qkv, b_qkv, w_proj, b_proj)
    return out


# revision 36
# speedup vs baseline: 1.1212x; 1.0079x over previous
"""Multi-head self-attention (B=4, T=2048, C=1024, H=16, causal) on 8 TRN2 cores.

Sharding: core = (batch b, head-group hg) with b in 0..3, hg in 0..1.
Each core computes its batch's QKV projection restricted to its 8 heads
(w_qkv column-sharded), full causal attention for those heads, and a
row-sharded partial of the output projection. Host sums the two partials
per batch and adds b_proj.

Dataflow is fully "transposed": activations live as [feature, token] so
every matmul contracts along SBUF partitions and softmax needs no
transposes — attention scores come out as S^T[key, query], softmax
normalization is folded into an extra all-ones column appended to V
(giving sum-of-exp for free), and the normalized O^T[feat, query] feeds
the output projection directly as its moving operand.
"""

import sys
from contextlib import ExitStack

for _p in ("/opt/trn_rl_repo",):
    if _p not in sys.path:
        sys.path.insert(0, _p)

import numpy as np

import concourse.bass as bass
import concourse.bacc as bacc
import concourse.tile as tile
from concourse import mybir
from concourse.bass_utils import run_bass_kernel_spmd

F32 = mybir.dt.float32

B, C, H, HD = 4, 1024, 16, 64
HPC = 8  # heads per core
N_CORES = 8
SCALE = HD ** -0.5
T_FULL = 2048

# Matmul input dtype: float32r streams fp32 operands at 1 cycle/row
# (vs 4 for plain fp32) when the free dim is >= 256.
MM_DT = mybir.dt.float32r


def build_program(T=T_FULL, mm_dt=MM_DT):
    NT = T // 512   # query chunks
    NKB = T // 128  # key blocks
    nc = bacc.Bacc("TRN2", target_bir_lowering=False)
    MD = mm_dt  # dtype for tensors feeding matmuls (float32r rounds at write)

    xT = nc.dram_tensor("xT", [C, T], MD, kind="ExternalInput")
    wqk = nc.dram_tensor("wqk", [C, 1024], MD, kind="ExternalInput")
    wv = nc.dram_tensor("wv", [C, 512], MD, kind="ExternalInput")
    bqk = nc.dram_tensor("bqk", [128, 8], F32, kind="ExternalInput")
    bvb = nc.dram_tensor("bvb", [128, 512], F32, kind="ExternalInput")
    wp = nc.dram_tensor("wp", [512, 1024], MD, kind="ExternalInput")
    masks = nc.dram_tensor("masks", [128, 2048], F32, kind="ExternalInput")
    outT = nc.dram_tensor("outT", [C, T], F32, kind="ExternalOutput")

    with nc.allow_low_precision("float32r matmul pipeline"), \
         tile.TileContext(nc) as tc, ExitStack() as ctx:
        pers = ctx.enter_context(tc.tile_pool(name="pers", bufs=1))
        KT_sb = pers.tile([128, 4 * T], MD, tag="kt", name="KT_sb")
        V_sb = pers.tile([128, NKB * 520], MD, tag="v", name="V_sb")
        Wqk_sb = pers.tile([128, 8 * 1024], MD, tag="wqk", name="Wqk_sb")
        Wv_sb = pers.tile([128, 8 * 512], MD, tag="wv", name="Wv_sb")
        masks_sb = pers.tile([128, 2048], F32, tag="masks", name="masks_sb")
        bqk_sb = pers.tile([128, 8], F32, tag="bqk", name="bqk_sb")
        bvb_sb = pers.tile([128, 512], F32, tag="bvb", name="bvb_sb")
        # all-ones row at partition 64 (engine APs must start 32-aligned;
        # the sum-of-exp lane of attention psum lives at row 64)
        ones_sb = pers.tile([65, 64], F32, tag="ones", name="ones_sb")

        dram = ctx.enter_context(tc.tile_pool(name="dramp", bufs=1, space="DRAM"))
        qtds = [dram.tile([512, 512], MD, tag=f"qtd{t}", name=f"qtd{t}")
                for t in range(NT)]

        mmp = ctx.enter_context(tc.tile_pool(name="mmp", bufs=2, space="PSUM"))
        sp = ctx.enter_context(tc.tile_pool(name="sp", bufs=1, space="PSUM"))
        op = ctx.enter_context(tc.tile_pool(name="op", bufs=1, space="PSUM"))

        xtp = ctx.enter_context(tc.tile_pool(name="xtp", bufs=1))
        qbp = ctx.enter_context(tc.tile_pool(name="qbp", bufs=2))
        qsp = ctx.enter_context(tc.tile_pool(name="qsp", bufs=1))
        otp = ctx.enter_context(tc.tile_pool(name="otp", bufs=2))
        wpp = ctx.enter_context(tc.tile_pool(name="wpp", bufs=3))
        esp = ctx.enter_context(tc.tile_pool(name="esp", bufs=2))
        rsp = ctx.enter_context(tc.tile_pool(name="rsp", bufs=2))
        obp = ctx.enter_context(tc.tile_pool(name="obp", bufs=2))

        nc.vector.memset(ones_sb[:, :], 1.0)
        # col 64 of each 65-wide V head slot must be 1.0 (sum-of-exp lane);
        # fill all of V with 1.0, V evictions overwrite cols 0-63
        nc.vector.memset(V_sb[:, :].bitcast(F32), 1.0)

        nc.sync.dma_start(out=masks_sb[:, :], in_=masks[:, :])
        nc.sync.dma_start(out=bqk_sb[:, :], in_=bqk[:, :])
        nc.sync.dma_start(out=bvb_sb[:, :], in_=bvb[:, :])
        for c in range(8):
            nc.sync.dma_start(
                out=Wqk_sb[:, c * 1024:(c + 1) * 1024],
                in_=wqk[c * 128:(c + 1) * 128, :])
            nc.sync.dma_start(
                out=Wv_sb[:, c * 512:(c + 1) * 512],
                in_=wv[c * 128:(c + 1) * 128, :])

        xT3 = xT.rearrange("(c p) t -> p c t", p=128)
        wp3 = wp.rearrange("(cp p) n -> p cp n", p=128)

        def load_chunk(t):
            xt_all = xtp.tile([128, 8, 512], MD, tag="xt", name="xt_all")
            nc.sync.dma_start(
                out=xt_all[:, :, :], in_=xT3[:, :, t * 512:(t + 1) * 512])
            return xt_all

        def qk_chain(t, m, xt_all):
            ps = mmp.tile([128, 512], F32, tag="mm", name="ps_qk")
            for c in range(8):
                nc.tensor.matmul(
                    out=ps[:, :],
                    lhsT=Wqk_sb[:, c * 1024 + m * 128: c * 1024 + (m + 1) * 128],
                    rhs=xt_all[:, c, :],
                    start=(c == 0), stop=(c == 7))
            if m < 4:
                qb = qbp.tile([128, 512], MD, tag="qb", name="qb")
                nc.vector.tensor_scalar_add(
                    out=qb[:, :], in0=ps[:, :], scalar1=bqk_sb[:, m:m + 1])
                nc.sync.dma_start(
                    out=qtds[t][m * 128:(m + 1) * 128, :], in_=qb[:, :])
            else:
                nc.vector.tensor_scalar_add(
                    out=KT_sb[:, (m - 4) * T + t * 512: (m - 4) * T + (t + 1) * 512],
                    in0=ps[:, :], scalar1=bqk_sb[:, m:m + 1])

        def v_chain(t, tb, xt_all):
            kbg = t * 4 + tb
            ps = mmp.tile([128, 512], F32, tag="mm", name="ps_v")
            for c in range(8):
                nc.tensor.matmul(
                    out=ps[:, :],
                    lhsT=xt_all[:, c, tb * 128:(tb + 1) * 128],
                    rhs=Wv_sb[:, c * 512:(c + 1) * 512],
                    start=(c == 0), stop=(c == 7))
            vdst = V_sb[:, kbg * 520:(kbg + 1) * 520].rearrange(
                "p (h e) -> p h e", e=65)[:, :, 0:64]
            nc.vector.scalar_tensor_tensor(
                out=vdst,
                in0=ps.rearrange("p (h e) -> p h e", e=64),
                scalar=1.0,
                in1=bvb_sb.rearrange("p (h e) -> p h e", e=64),
                op0=mybir.AluOpType.mult,
                op1=mybir.AluOpType.add)

        def chunk_chains(t):
            xt_all = load_chunk(t)
            work = []
            for m in range(8):
                work.append(lambda m=m: qk_chain(t, m, xt_all))
            for tb in range(4):
                work.append(lambda tb=tb: v_chain(t, tb, xt_all))
            return work

        def outproj_chain(j, OT_j, nb):
            wpt = wpp.tile([128, 4, 128], MD, tag="wpt", name="wpt")
            nc.sync.dma_start(
                out=wpt[:, :, :], in_=wp3[:, :, nb * 128:(nb + 1) * 128])
            ps = mmp.tile([128, 512], F32, tag="mm", name="ps_o")
            for cp in range(4):
                nc.tensor.matmul(
                    out=ps[:, :], lhsT=wpt[:, cp, :], rhs=OT_j[:, cp, :],
                    start=(cp == 0), stop=(cp == 3))
            ob = obp.tile([128, 512], F32, tag="ob", name="ob")
            nc.vector.tensor_copy(out=ob[:, :], in_=ps[:, :])
            nc.sync.dma_start(
                out=outT[nb * 128:(nb + 1) * 128, j * 512:(j + 1) * 512],
                in_=ob[:, :])

        def outproj_chains(j, OT_j):
            return [lambda nb=nb: outproj_chain(j, OT_j, nb) for nb in range(8)]

        # ---- prime: projection for query/key chunk 0 ----
        for ch in chunk_chains(0):
            ch()

        # ---- supersteps: attention(j) interleaved with projection(j+1),
        #      output projection(j) as the dense tail of each superstep ----
        prev_ot = None
        for j in range(NT):
            qst = qsp.tile([128, 4, 512], MD, tag="qs", name="qs")
            nc.sync.dma_start(
                out=qst[:, :, :],
                in_=qtds[j].rearrange("(p r) t -> r p t", r=128))
            work = list(chunk_chains(j + 1)) if j + 1 < NT else []
            if prev_ot is not None:
                work += outproj_chains(j - 1, prev_ot)
            wi = 0
            gi = 0
            OT_j = otp.tile([128, 4, 512], MD, tag="ot", name="OT_j")
            for p in range(4):
                rstg = rsp.tile([65, 1024], F32, tag="rstg", name="rstg")
                nkb = 4 * (j + 1)
                poa = op.tile([65, 512], F32, tag="oa", name="poa")
                pob = op.tile([65, 512], F32, tag="ob2", name="pob")
                pos = (poa, pob)
                for G in range(2 * (j + 1)):
                    pssa = sp.tile([128, 1024], F32, tag="sa", name="pssa")
                    pssb = sp.tile([128, 1024], F32, tag="sb", name="pssb")
                    psss = (pssa, pssb)
                    for r in range(2):
                        kb = 2 * G + r
                        # heads of the pair occupy PE row groups 0-63/64-127
                        for parity in range(2):
                            nc.tensor.matmul(
                                out=psss[parity][:, r * 512:(r + 1) * 512],
                                lhsT=KT_sb[64 * parity:64 * (parity + 1),
                                           p * T + kb * 128: p * T + (kb + 1) * 128],
                                rhs=qst[64 * parity:64 * (parity + 1), p, :],
                                start=True, stop=True)
                    ests = []
                    for parity in range(2):
                        est = esp.tile([128, 1024], MD, tag=f"es{parity}",
                                       name=f"est{parity}")
                        nc.scalar.activation(
                            out=est[:, :], in_=psss[parity][:, :],
                            func=mybir.ActivationFunctionType.Exp, scale=SCALE)
                        if G >= 2 * j:
                            nc.vector.tensor_mul(
                                out=est[:, :], in0=est[:, :],
                                in1=masks_sb[:, (G - 2 * j) * 1024:(G - 2 * j + 1) * 1024])
                        ests.append(est)
                    for r in range(2):
                        kb = 2 * G + r
                        for parity in range(2):
                            h = 2 * p + parity
                            nc.tensor.matmul(
                                out=pos[parity][:, :],
                                lhsT=V_sb[:, kb * 520 + h * 65: kb * 520 + (h + 1) * 65],
                                rhs=ests[parity][:, r * 512:(r + 1) * 512],
                                start=(kb == 0), stop=(kb == nkb - 1))
                    # keep the PE fed with dense projection work; paired
                    # chains give ~3.8us uninterrupted bursts, enough to
                    # flip the HAM clock gate back to full rate
                    gi += 1
                    if gi % 2 == 0:
                        for _ in range(2):
                            if wi < len(work):
                                work[wi]()
                                wi += 1
                for parity in range(2):
                    po = pos[parity]
                    # 1/sum_exp for this head (psum row 64 is 32-aligned)
                    nc.vector.reciprocal(
                        out=rstg[64:65, parity * 512:(parity + 1) * 512],
                        in_=po[64:65, :])
                    if parity == 0:
                        nc.vector.tensor_copy(
                            out=OT_j[0:64, p, :], in_=po[0:64, :])
                    else:
                        # odd head rows live at partitions 64-127; engines
                        # cannot partition-shift, bounce via SBUF + DMA
                        stg = obp.tile([64, 512], MD, tag="stg", name="stg")
                        nc.vector.tensor_copy(out=stg[:, :], in_=po[0:64, :])
                        nc.sync.dma_start(
                            out=OT_j[64:128, p, :], in_=stg[:, :])
                # broadcast each head's 1/sum_exp over its 64 O^T rows
                rb = mmp.tile([128, 512], F32, tag="mm", name="rb")
                for parity in range(2):
                    nc.tensor.matmul(
                        out=rb[64 * parity:64 * (parity + 1), :],
                        lhsT=ones_sb[64:65, :],
                        rhs=rstg[64:65, parity * 512:(parity + 1) * 512],
                        start=True, stop=True)
                otn = OT_j[:, p, :]
                nc.vector.tensor_mul(out=otn, in0=otn, in1=rb[:, :])
            while wi < len(work):
                work[wi]()
                wi += 1
            prev_ot = OT_j
        for ch in outproj_chains(NT - 1, prev_ot):
            ch()
    nc.compile()
    return nc


def build_masks():
    # masks[kp, r*512 + qf] = 1.0 iff qf >= kp + 128*r  (r = key-block index
    # within the diagonal group of a 512-wide query chunk)
    kp = np.arange(128)[:, None]
    qf = np.arange(512)[None, :]
    cols = [(qf >= kp + 128 * r).astype(np.float32) for r in range(4)]
    return np.ascontiguousarray(np.concatenate(cols, axis=1))


def make_in_maps(x, w_qkv, b_qkv, w_proj, T=T_FULL):
    x = np.asarray(x, dtype=np.float32)
    w_qkv = np.asarray(w_qkv, dtype=np.float32)
    b_qkv = np.asarray(b_qkv, dtype=np.float32)
    w_proj = np.asarray(w_proj, dtype=np.float32)
    masks = build_masks()
    in_maps = []
    for core in range(N_CORES):
        b, hg = core // 2, core % 2
        qc = slice(hg * 512, hg * 512 + 512)
        kc = slice(1024 + hg * 512, 1024 + hg * 512 + 512)
        vc = slice(2048 + hg * 512, 2048 + hg * 512 + 512)
        bqk_host = np.concatenate([b_qkv[qc], b_qkv[kc]]).reshape(8, 128).T
        in_maps.append({
            "xT": np.ascontiguousarray(x[b, :T, :].T),
            "wqk": np.ascontiguousarray(
                np.concatenate([w_qkv[:, qc], w_qkv[:, kc]], axis=1)),
            "wv": np.ascontiguousarray(w_qkv[:, vc]),
            "bqk": np.ascontiguousarray(bqk_host),
            "bvb": np.ascontiguousarray(
                np.broadcast_to(b_qkv[vc], (128, 512))),
            "wp": np.ascontiguousarray(w_proj[hg * 512:(hg + 1) * 512, :]),
            "masks": masks,
        })
    return in_maps


def assemble_output(results, b_proj, T=T_FULL):
    b_proj = np.asarray(b_proj, dtype=np.float32)
    out = np.empty((B, T, C), dtype=np.float32)
    for b in range(B):
        acc = results[2 * b]["outT"] + results[2 * b + 1]["outT"]
        out[b] = acc.T + b_proj
    return out


_PROG_CACHE = {}


def run(x, w_qkv, b_qkv, w_proj, b_proj, T=T_FULL, mm_dt=MM_DT, trace=False):
    key = (T, str(mm_dt))
    if key not in _PROG_CACHE:
        _PROG_CACHE[key] = build_program(T=T, mm_dt=mm_dt)
    nc = _PROG_CACHE[key]
    in_maps = make_in_maps(x, w_qkv, b_qkv, w_proj, T=T)
    res = run_bass_kernel_spmd(
        nc, in_maps, list(range(N_CORES)), trace=trace,
    )
    out = assemble_output(res.results, b_proj, T=T)
    return out, res


def kernel(x, w_qkv, b_qkv, w_proj, b_proj):
    out, _ = run(x, w_qkv, b_qkv, w_proj, b_proj)
    return out
